# revision 1
# baseline (speedup 1.0000x reference)
import functools

import jax
import jax.numpy as jnp
import numpy as np

# Problem constants (nn_C2LMGAgent): hardcoded per harness contract.
BZ, NA, D, R, H, IN, NACT = 64, 64, 128, 3, 4, 96, 20
M = 8  # cores / data-parallel shards over batch dim


def _forward(obs, hidden, gumbel, W_enc, b_enc, W_ih, W_hh, b_ih, b_hh,
             W_oth, b_oth, W_m1, b_m1, W_m2, b_m2, W_s1, b_s1, W_s2, b_s2,
             Wq, Wk, Wv, Wo, W_q1, b_q1, W_q2, b_q2):
    """Per-shard forward. obs:(bz,na,IN) hidden:(bz,na,D) gumbel:(bz,R,na,na,2)."""
    bz, na, _ = obs.shape
    d = W_enc.shape[0]
    x = obs @ W_enc.T + b_enc
    gx = x @ W_ih.T + b_ih
    gh = hidden @ W_hh.T + b_hh
    xr, xz, xn = jnp.split(gx, 3, -1)
    hr, hz, hn = jnp.split(gh, 3, -1)
    r = jax.nn.sigmoid(xr + hr)
    z = jax.nn.sigmoid(xz + hz)
    nc = jnp.tanh(xn + r * hn)
    h = (1.0 - z) * nc + z * hidden
    others = h @ W_oth.T + b_oth
    others_info = jnp.broadcast_to(others[:, None, :, :], (bz, na, na, d))
    self_info = jnp.broadcast_to(h[:, :, None, :], (bz, na, na, d))
    msg_in = jnp.concatenate([others_info, self_info], -1)
    message = jax.nn.relu(jax.nn.relu(msg_in @ W_m1.T + b_m1) @ W_m2.T + b_m2)
    sched_in = jnp.concatenate([self_info, message], -1)
    nH, dk = H, d // H
    scale = 1.0 / np.sqrt(dk)
    results = []
    for i in range(R):
        def sched_mlp(xx):
            return jax.nn.relu(xx @ W_s1[i].T + b_s1[i]) @ W_s2[i].T + b_s2[i]
        logits = 0.5 * sched_mlp(sched_in) + 0.5 * sched_mlp(jnp.swapaxes(sched_in, 1, 2))
        y = logits + gumbel[:, i]
        y_soft = jax.nn.softmax(y, -1)
        y_hard = jax.nn.one_hot(jnp.argmax(y, -1), 2, dtype=y_soft.dtype)
        y_st = y_hard + y_soft - jax.lax.stop_gradient(y_soft)
        adj = y_st[..., 1]
        q = (h @ Wq[i].T).reshape(bz, na, nH, dk)
        k = (message @ Wk[i].T).reshape(bz, na, na, nH, dk)
        v = (message @ Wv[i].T).reshape(bz, na, na, nH, dk)
        scores = jnp.einsum('bihe,bijhe->bijh', q, k) * scale
        w = jax.nn.softmax(scores, axis=2) * adj[..., None]
        w = w / (jnp.sum(w, axis=2, keepdims=True) + 1e-10)
        att = jnp.einsum('bijh,bijhe->bihe', w, v).reshape(bz, na, d) @ Wo[i].T
        results.append(jax.nn.elu(att))
    aggre = jnp.mean(jnp.stack(results), 0)
    qout = jnp.concatenate([h, aggre], -1)
    qout = jax.nn.relu(qout @ W_q1.T + b_q1) @ W_q2.T + b_q2
    return qout, h


_WNAMES = ["W_enc", "b_enc", "W_ih", "W_hh", "b_ih", "b_hh", "W_oth", "b_oth",
           "W_m1", "b_m1", "W_m2", "b_m2", "W_s1", "b_s1", "W_s2", "b_s2",
           "Wq", "Wk", "Wv", "Wo", "W_q1", "b_q1", "W_q2", "b_q2"]

_pmapped = None


def _get_pmapped():
    global _pmapped
    if _pmapped is None:
        fn = functools.partial(_forward)

        def shard_fn(obs, hidden, gumbel, weights):
            with jax.default_matmul_precision("float32"):
                return _forward(obs, hidden, gumbel, *weights)

        _pmapped = jax.pmap(shard_fn, in_axes=(0, 0, 0, None), devices=jax.devices()[:M])
    return _pmapped


def kernel(**inputs):
    obs = np.asarray(inputs["obs"], np.float32)
    hidden = np.asarray(inputs["hidden"], np.float32)
    gumbel = np.asarray(inputs["gumbel"], np.float32)

    # Shard batch dim across the 8 NeuronCores; replicate weights.
    sb = BZ // M
    obs_s = obs.reshape(M, sb, NA, IN)
    hid_s = hidden.reshape(M, sb, NA, D)
    # gumbel (R,BZ,NA,NA,2) -> (M, sb, R, NA, NA, 2)
    gum_s = np.ascontiguousarray(np.transpose(gumbel, (1, 0, 2, 3, 4))).reshape(
        M, sb, R, NA, NA, 2)

    weights = tuple(np.asarray(inputs[n], np.float32) for n in _WNAMES)
    qout_s, h_s = _get_pmapped()(obs_s, hid_s, gum_s, weights)
    qout = np.asarray(qout_s).reshape(BZ, NA, NACT)
    h = np.asarray(h_s).reshape(BZ, NA, D)
    return qout, h


# revision 3
# speedup vs baseline: 11.1194x; 11.1194x over previous
import functools

import jax
import jax.numpy as jnp
import numpy as np

# Problem constants (nn_C2LMGAgent): hardcoded per harness contract.
BZ, NA, D, R, H, IN, NACT = 64, 64, 128, 3, 4, 96, 20
M = 8  # cores / data-parallel shards over batch dim


def _forward(obs, hidden, gumbel, W_enc, b_enc, W_ih, W_hh, b_ih, b_hh,
             W_oth, b_oth, W_m1, b_m1, W_m2, b_m2, W_s1, b_s1, W_s2, b_s2,
             Wq, Wk, Wv, Wo, W_q1, b_q1, W_q2, b_q2):
    """Per-shard forward. obs:(bz,na,IN) hidden:(bz,na,D) gumbel:(bz,R,na,na,2)."""
    bz, na, _ = obs.shape
    d = W_enc.shape[0]
    x = obs @ W_enc.T + b_enc
    gx = x @ W_ih.T + b_ih
    gh = hidden @ W_hh.T + b_hh
    xr, xz, xn = jnp.split(gx, 3, -1)
    hr, hz, hn = jnp.split(gh, 3, -1)
    r = jax.nn.sigmoid(xr + hr)
    z = jax.nn.sigmoid(xz + hz)
    nc = jnp.tanh(xn + r * hn)
    h = (1.0 - z) * nc + z * hidden
    others = h @ W_oth.T + b_oth
    others_info = jnp.broadcast_to(others[:, None, :, :], (bz, na, na, d))
    self_info = jnp.broadcast_to(h[:, :, None, :], (bz, na, na, d))
    msg_in = jnp.concatenate([others_info, self_info], -1)
    message = jax.nn.relu(jax.nn.relu(msg_in @ W_m1.T + b_m1) @ W_m2.T + b_m2)
    sched_in = jnp.concatenate([self_info, message], -1)
    nH, dk = H, d // H
    scale = 1.0 / np.sqrt(dk)
    results = []
    for i in range(R):
        def sched_mlp(xx):
            return jax.nn.relu(xx @ W_s1[i].T + b_s1[i]) @ W_s2[i].T + b_s2[i]
        logits = 0.5 * sched_mlp(sched_in) + 0.5 * sched_mlp(jnp.swapaxes(sched_in, 1, 2))
        y = logits + gumbel[:, i]
        y_soft = jax.nn.softmax(y, -1)
        y_hard = jax.nn.one_hot(jnp.argmax(y, -1), 2, dtype=y_soft.dtype)
        y_st = y_hard + y_soft - jax.lax.stop_gradient(y_soft)
        adj = y_st[..., 1]
        q = (h @ Wq[i].T).reshape(bz, na, nH, dk)
        k = (message @ Wk[i].T).reshape(bz, na, na, nH, dk)
        v = (message @ Wv[i].T).reshape(bz, na, na, nH, dk)
        scores = jnp.einsum('bihe,bijhe->bijh', q, k) * scale
        w = jax.nn.softmax(scores, axis=2) * adj[..., None]
        w = w / (jnp.sum(w, axis=2, keepdims=True) + 1e-10)
        att = jnp.einsum('bijh,bijhe->bihe', w, v).reshape(bz, na, d) @ Wo[i].T
        results.append(jax.nn.elu(att))
    aggre = jnp.mean(jnp.stack(results), 0)
    qout = jnp.concatenate([h, aggre], -1)
    qout = jax.nn.relu(qout @ W_q1.T + b_q1) @ W_q2.T + b_q2
    return qout, h


_WNAMES = ["W_enc", "b_enc", "W_ih", "W_hh", "b_ih", "b_hh", "W_oth", "b_oth",
           "W_m1", "b_m1", "W_m2", "b_m2", "W_s1", "b_s1", "W_s2", "b_s2",
           "Wq", "Wk", "Wv", "Wo", "W_q1", "b_q1", "W_q2", "b_q2"]

_pmapped = None


def _get_pmapped():
    global _pmapped
    if _pmapped is None:
        fn = functools.partial(_forward)

        def shard_fn(obs, hidden, gumbel, weights):
            with jax.default_matmul_precision("float32"):
                return _forward(obs, hidden, gumbel, *weights)

        _pmapped = jax.pmap(shard_fn, in_axes=(0, 0, 0, 0), devices=jax.devices()[:M])
    return _pmapped


def kernel(**inputs):
    obs = np.asarray(inputs["obs"], np.float32)
    hidden = np.asarray(inputs["hidden"], np.float32)
    gumbel = np.asarray(inputs["gumbel"], np.float32)

    # Shard batch dim across the 8 NeuronCores; replicate weights.
    sb = BZ // M
    obs_s = obs.reshape(M, sb, NA, IN)
    hid_s = hidden.reshape(M, sb, NA, D)
    # gumbel (R,BZ,NA,NA,2) -> (M, sb, R, NA, NA, 2)
    gum_s = np.ascontiguousarray(np.transpose(gumbel, (1, 0, 2, 3, 4))).reshape(
        M, sb, R, NA, NA, 2)

    weights = tuple(
        np.broadcast_to(np.asarray(inputs[n], np.float32), (M,) + inputs[n].shape)
        for n in _WNAMES)
    qout_s, h_s = _get_pmapped()(obs_s, hid_s, gum_s, weights)
    qout = np.asarray(qout_s).reshape(BZ, NA, NACT)
    h = np.asarray(h_s).reshape(BZ, NA, D)
    return qout, h


# revision 4
# speedup vs baseline: 19.3084x; 1.7365x over previous
import functools

import jax
import jax.numpy as jnp
import numpy as np

# Problem constants (nn_C2LMGAgent): hardcoded per harness contract.
BZ, NA, D, R, H, IN, NACT = 64, 64, 128, 3, 4, 96, 20
M = 8  # cores / data-parallel shards over batch dim


def _forward(obs, hidden, gumbel, W_enc, b_enc, W_ih, W_hh, b_ih, b_hh,
             W_oth, b_oth, W_m1, b_m1, W_m2, b_m2, W_s1, b_s1, W_s2, b_s2,
             Wq, Wk, Wv, Wo, W_q1, b_q1, W_q2, b_q2):
    """Per-shard forward. obs:(bz,na,IN) hidden:(bz,na,D) gumbel:(bz,R,na,na,2)."""
    bz, na, _ = obs.shape
    d = W_enc.shape[0]
    x = obs @ W_enc.T + b_enc
    gx = x @ W_ih.T + b_ih
    gh = hidden @ W_hh.T + b_hh
    xr, xz, xn = jnp.split(gx, 3, -1)
    hr, hz, hn = jnp.split(gh, 3, -1)
    r = jax.nn.sigmoid(xr + hr)
    z = jax.nn.sigmoid(xz + hz)
    nc = jnp.tanh(xn + r * hn)
    h = (1.0 - z) * nc + z * hidden
    others = h @ W_oth.T + b_oth
    # msg_in = [others_j, h_i]: split W_m1 and broadcast-add instead of concat
    A = others @ W_m1[:, :d].T                     # (bz, j, d)
    B = h @ W_m1[:, d:].T + b_m1                   # (bz, i, d)
    L1 = jax.nn.relu(A[:, None, :, :] + B[:, :, None, :])   # (bz,i,j,d)
    message = jax.nn.relu(L1 @ W_m2.T + b_m2)
    nH, dk = H, d // H
    scale = 1.0 / np.sqrt(dk)
    results = []
    for i in range(R):
        # sched_in = [h_i, message_ij]; f(swap(x)) = swap(f(x)) for the
        # elementwise MLP, so compute F once and symmetrize.
        C = h @ W_s1[i][:, :d].T + b_s1[i]         # (bz, i, 64)
        Dm = message @ W_s1[i][:, d:].T            # (bz, i, j, 64)
        F = jax.nn.relu(C[:, :, None, :] + Dm) @ W_s2[i].T + b_s2[i]
        logits = 0.5 * F + 0.5 * jnp.swapaxes(F, 1, 2)
        y = logits + gumbel[:, i]
        adj = (y[..., 1] > y[..., 0]).astype(jnp.float32)
        q = (h @ Wq[i].T).reshape(bz, na, nH, dk)
        # scores = q . (Wk msg) = msg . (Wk_h^T q_h): fold Wk out of pair dim
        Wk_r = Wk[i].reshape(nH, dk, d)
        u = jnp.einsum('hed,bihe->bihd', Wk_r, q)
        scores = jnp.einsum('bijd,bihd->bijh', message, u) * scale
        w = jax.nn.softmax(scores, axis=2) * adj[..., None]
        w = w / (jnp.sum(w, axis=2, keepdims=True) + 1e-10)
        # att = sum_j w * (Wv msg): weighted-sum msg first, project after
        mw = jnp.einsum('bijh,bijd->bihd', w, message)
        Wv_r = Wv[i].reshape(nH, dk, d)
        att = jnp.einsum('bihd,hed->bihe', mw, Wv_r).reshape(bz, na, d) @ Wo[i].T
        results.append(jax.nn.elu(att))
    aggre = jnp.mean(jnp.stack(results), 0)
    qout = jnp.concatenate([h, aggre], -1)
    qout = jax.nn.relu(qout @ W_q1.T + b_q1) @ W_q2.T + b_q2
    return qout, h


_WNAMES = ["W_enc", "b_enc", "W_ih", "W_hh", "b_ih", "b_hh", "W_oth", "b_oth",
           "W_m1", "b_m1", "W_m2", "b_m2", "W_s1", "b_s1", "W_s2", "b_s2",
           "Wq", "Wk", "Wv", "Wo", "W_q1", "b_q1", "W_q2", "b_q2"]

_pmapped = None


def _get_pmapped():
    global _pmapped
    if _pmapped is None:
        fn = functools.partial(_forward)

        def shard_fn(obs, hidden, gumbel, weights):
            with jax.default_matmul_precision("float32"):
                return _forward(obs, hidden, gumbel, *weights)

        _pmapped = jax.pmap(shard_fn, in_axes=(0, 0, 0, 0), devices=jax.devices()[:M])
    return _pmapped


def kernel(**inputs):
    obs = np.asarray(inputs["obs"], np.float32)
    hidden = np.asarray(inputs["hidden"], np.float32)
    gumbel = np.asarray(inputs["gumbel"], np.float32)

    # Shard batch dim across the 8 NeuronCores; replicate weights.
    sb = BZ // M
    obs_s = obs.reshape(M, sb, NA, IN)
    hid_s = hidden.reshape(M, sb, NA, D)
    # gumbel (R,BZ,NA,NA,2) -> (M, sb, R, NA, NA, 2)
    gum_s = np.ascontiguousarray(np.transpose(gumbel, (1, 0, 2, 3, 4))).reshape(
        M, sb, R, NA, NA, 2)

    weights = tuple(
        np.broadcast_to(np.asarray(inputs[n], np.float32), (M,) + inputs[n].shape)
        for n in _WNAMES)
    qout_s, h_s = _get_pmapped()(obs_s, hid_s, gum_s, weights)
    qout = np.asarray(qout_s).reshape(BZ, NA, NACT)
    h = np.asarray(h_s).reshape(BZ, NA, D)
    return qout, h
